# revision 1
# baseline (speedup 1.0000x reference)
"""Longformer block on 8 TRN2 NeuronCores (Bass/Tile, SPMD).

Sharding: data-parallel over (batch, sequence): core c -> batch c//4, token
chunk (c%4)*512..+512. Weights replicated (bf16). Everything on-chip stays in
transposed [D, token] layout so LN/residual/matmuls need no device transposes
(host pre-transposes x; LN stats via ones-vector matmuls on PE).

Attention: banded causal window (halo of 128 tokens recomputed locally) + the
token-0 global column as a 257th score column. The one global *row* (token
T-1 attends everything) is computed via per-core exp-sum partials over each
core's own K/V slice, combined with a tiny in-kernel AllReduce (each core
deposits its partial into its batch's block, scaled by 0/1 flag inputs), and
patched into the owning core's output column with copy_predicated.
"""

import numpy as np
import ml_dtypes

import concourse.bass as bass
import concourse.mybir as mybir
import concourse.tile as tile
from concourse.masks import make_identity
from concourse.bass_utils import run_bass_kernel_spmd

F32 = mybir.dt.float32
BF16 = mybir.dt.bfloat16
AF = mybir.ActivationFunctionType
ALU = mybir.AluOpType
AX = mybir.AxisListType

D = 1024
H = 16
HD = 64
T = 2048
B = 2
CHUNK = 512
HALO = 128
NSLOT = 768          # [halo 128 | own 512 | t0 | t2047 | pad]
NKV = 641            # slots 0..640 hold K/V (640 = token0); 641 = q2047 src
NQB = 4
WIN = 256
NEG = -1e30
EPS = 1e-5
N_CORES = 8
SKIP_CC = [False]   # set kernel.SKIP_CC[0]=True to build without the
                    # collective (TimelineSim is single-core only)
PHASE_MARKS = []    # (phase_name, first_inst_id) filled during _emit


def _mark(nc, name):
    PHASE_MARKS.append((name, set(nc.inst_map.keys())))

# ---------------------------------------------------------------- bir fix ---

_waitfix_ctr = [0]


def _split_multiwaits(nc):
    """This container's walrus accepts ONE sync-wait per instruction; Tile
    attaches several. Hoist extras onto NoOps just before each instruction
    (Tile sems are monotonic within a context, so sequential waits are
    equivalent)."""
    n = 0
    for func in nc.m.functions:
        for bb in func.blocks:
            out = []
            changed = False
            for inst in bb.instructions:
                si = inst.sync_info
                if si is not None and len(si.on_wait) > 1:
                    waits = list(si.on_wait)
                    keep = [w for w in waits
                            if getattr(w, "wait_mode", "") not in
                            ("sem-ge-imm", "sem-ge-reg")]
                    if keep:
                        hoist = [w for w in waits if w not in keep]
                        last = keep
                    else:
                        hoist, last = waits[:-1], [waits[-1]]
                    for w in hoist:
                        _waitfix_ctr[0] += 1
                        nop = mybir.InstNoOp(name=f"I-waitfix-{_waitfix_ctr[0]}")
                        nop.engine = inst.engine
                        nop.sync_info = mybir.SyncInfo(on_wait=[w], on_update=[])
                        out.append(nop)
                        n += 1
                    si.on_wait = last
                    changed = True
                out.append(inst)
            if changed:
                bb.instructions[:] = out
    return n

# ------------------------------------------------------------ host helpers --


def _make_x_ext(x, c):
    b, j = divmod(c, 4)
    start = j * CHUNK
    ext = np.zeros((NSLOT, D), np.float32)
    ext[0:HALO] = x[b, start - HALO:start] if j > 0 else x[b, 0:HALO]
    ext[HALO:HALO + CHUNK] = x[b, start:start + CHUNK]
    ext[640] = x[b, 0]
    ext[641] = x[b, T - 1]
    return ext


def _make_mask(c):
    b, j = divmod(c, 4)
    start = j * CHUNK
    m = np.full((NQB, 128, WIN + 1), NEG, np.float32)
    il = np.arange(128)[:, None]
    jl = np.arange(WIN)[None, :]
    for qb in range(NQB):
        q_abs = start + qb * 128 + il
        slot = qb * 128 + jl
        band = (jl >= il) & (jl <= il + 128)
        valid = (j > 0) | (slot >= HALO)
        blk = m[qb, :, :WIN]
        blk[band & valid] = 0.0
        tok0_in_band = (q_abs[:, 0] <= HALO) & (j == 0)
        m[qb, :, WIN] = np.where(tok0_in_band, NEG, 0.0)
    return m


def _tileP(a, p=128):
    """[N*p, ...] -> [p, N, ...] partition-tiled layout."""
    n = a.shape[0] // p
    return np.ascontiguousarray(
        a.reshape(n, p, *a.shape[1:]).transpose(1, 0, *range(2, a.ndim + 1)))


def _vec_t(v):
    return np.ascontiguousarray(np.asarray(v, np.float32).reshape(-1, 128).T)

# ------------------------------------------------------------ bass program --


def _build_nc():
    nc = bass.Bass()

    inp = {}
    for name, shape, dt in [
        ("xT", [128, 8, NSLOT], F32),
        ("wq", [128, 8, D], BF16), ("wk", [128, 8, D], BF16),
        ("wv", [128, 8, D], BF16), ("wo", [128, 8, D], BF16),
        ("w1", [128, 32, 8, 128], BF16), ("w2", [32, 128, D], BF16),
        ("msk", [128, NQB, WIN + 1], BF16),
        ("g1T", [128, 8], F32), ("b1T", [128, 8], F32),
        ("g2T", [128, 8], F32), ("b2T", [128, 8], F32),
        ("boT", [128, 8], F32), ("bo2T", [128, 8], F32),
        ("b1h", [128, 32], F32),
        ("fixsel", [128, 1], mybir.dt.uint8),
        ("fA", [16, 1], F32), ("fB", [16, 1], F32),
    ]:
        inp[name] = nc.dram_tensor(name, shape, dt, kind="ExternalInput")
    out_d = nc.dram_tensor("outT", [128, 8, CHUNK], F32, kind="ExternalOutput")
    pin = nc.dram_tensor("pin", [H, 2, HD + 1], F32)
    pout = nc.dram_tensor("pout", [H, 2, HD + 1], F32, addr_space="Shared")
    ht_d = nc.dram_tensor("ht_d", [32, 128, CHUNK], BF16)

    with tile.TileContext(nc) as tc:
        _emit(nc, tc, inp, out_d, pin, pout, ht_d)
    _split_multiwaits(nc)
    return nc


def _emit(nc, tc, inp, out_d, pin, pout, ht_d):
    from contextlib import ExitStack
    ctx = ExitStack()
    with ctx:
        pers = ctx.enter_context(tc.tile_pool(name="pers", bufs=1))
        small = ctx.enter_context(tc.tile_pool(name="small", bufs=3))
        big = ctx.enter_context(tc.tile_pool(name="big", bufs=1))

        # ---- persistent constants / params
        idf = pers.tile([128, 128], F32, tag="idf")
        make_identity(nc, idf)
        idb = pers.tile([128, 128], BF16, tag="idb")
        make_identity(nc, idb)
        onesD = pers.tile([128, 1], BF16, tag="onesD")   # 1/D for means
        nc.vector.memset(onesD, 1.0 / D)
        onesb = pers.tile([128, 1], BF16, tag="onesb")
        nc.vector.memset(onesb, 1.0)
        ones1f = pers.tile([1, 128], F32, tag="ones1f")
        nc.vector.memset(ones1f, 1.0)
        epst = pers.tile([1, 1], F32, tag="epst")
        nc.vector.memset(epst, EPS)
        neg3 = pers.tile([128, 1], F32, tag="neg3")
        nc.vector.memset(neg3, -3.0)

        params = {}
        for nm in ["g1T", "b1T", "g2T", "b2T", "boT", "bo2T", "b1h", "fixsel",
                   "fA", "fB"]:
            dt = mybir.dt.uint8 if nm == "fixsel" else F32
            t = pers.tile(list(inp[nm].shape), dt, tag=nm, name=nm)
            nc.sync.dma_start(out=t, in_=inp[nm][:])
            params[nm] = t
        msk = pers.tile([128, NQB, WIN + 1], BF16, tag="msk")
        nc.sync.dma_start(out=msk, in_=inp["msk"][:])

        xT = big.tile([128, 8, NSLOT], F32, tag="xT")
        for kt in range(8):
            nc.sync.dma_start(out=xT[:, kt, :], in_=inp["xT"][:, kt, :])
        wvsb = big.tile([128, 8, D], BF16, tag="wv")
        nc.sync.dma_start(out=wvsb, in_=inp["wv"][:])
        hT = big.tile([128, 8, NSLOT], BF16, tag="hT")
        QT = big.tile([128, 8, CHUNK], BF16, tag="QT")
        q47T = big.tile([128, 8], BF16, tag="q47T")
        KT = big.tile([128, 8, NKV], BF16, tag="KT")
        V = big.tile([128, 6, D], BF16, tag="V")
        OT = big.tile([128, 8, CHUNK], BF16, tag="OT")
        yT = big.tile([128, 8, CHUNK], F32, tag="yT")
        h2T = big.tile([128, 8, CHUNK], BF16, tag="h2T")
        xbt = big.tile([128, 8, NSLOT], BF16, tag="ln_xb")

        # ================= LN in transposed layout =========================
        def layernorm_T(src, width, nchunks, g, b, out, pools):
            ps_row, ps_bc = pools
            cw = width // nchunks
            mus = []
            for cch in range(nchunks):
                mus.append((ps_row.tile([1, cw], F32, tag="row", name="mu"),
                            ps_row.tile([1, cw], F32, tag="row", name="msq")))
            for kt in range(8):
                xb = xbt[:, kt, 0:width]
                xsq = small.tile([128, width], BF16, tag="ln_xsq")
                nc.scalar.copy(out=xb, in_=src[:, kt, :])
                nc.scalar.square(out=xsq, in_=src[:, kt, :])
                for cch in range(nchunks):
                    sl = slice(cch * cw, (cch + 1) * cw)
                    nc.tensor.matmul(mus[cch][0], onesD, xb[:, sl],
                                     start=kt == 0, stop=kt == 7)
                    nc.tensor.matmul(mus[cch][1], onesD, xsq[:, sl],
                                     start=kt == 0, stop=kt == 7)
            bcs = []
            for cch in range(nchunks):
                sl = slice(cch * cw, (cch + 1) * cw)
                mu_ps, msq_ps = mus[cch]
                musb = small.tile([1, cw], F32, tag="ln_mu")
                nc.scalar.copy(out=musb, in_=mu_ps)
                tmp = small.tile([1, cw], F32, tag="ln_tmp")
                nc.vector.tensor_mul(out=tmp, in0=musb, in1=musb)
                nc.vector.tensor_sub(out=tmp, in0=msq_ps, in1=tmp)
                nc.scalar.activation(out=tmp, in_=tmp, func=AF.Sqrt,
                                     bias=epst, scale=1.0)
                nc.vector.reciprocal(out=tmp, in_=tmp)       # rstd
                nc.vector.tensor_mul(out=musb, in0=musb, in1=tmp)
                nc.scalar.mul(out=musb, in_=musb, mul=-1.0)  # -mu*rstd
                rb_ps = ps_bc.tile([128, cw], F32, tag="bc", name="rb")
                nc.tensor.matmul(rb_ps, ones1f, tmp, start=True, stop=True)
                nb_ps = ps_bc.tile([128, cw], F32, tag="bc", name="nb")
                nc.tensor.matmul(nb_ps, ones1f, musb, start=True, stop=True)
                rb_sb = small.tile([128, cw], BF16, tag="ln_rb")
                nc.scalar.copy(out=rb_sb, in_=rb_ps)
                nb_sb = small.tile([128, cw], BF16, tag="ln_nb")
                nc.scalar.copy(out=nb_sb, in_=nb_ps)
                bcs.append((rb_sb, nb_sb))
            for kt in range(8):
                for cch in range(nchunks):
                    sl = slice(cch * cw, (cch + 1) * cw)
                    rb_sb, nb_sb = bcs[cch]
                    t1 = small.tile([128, cw], BF16, tag="ln_t1")
                    nc.vector.tensor_mul(out=t1, in0=xbt[:, kt, sl], in1=rb_sb)
                    nc.vector.tensor_add(out=t1, in0=t1, in1=nb_sb)
                    nc.gpsimd.tensor_scalar(
                        out=out[:, kt, sl], in0=t1,
                        scalar1=g[:, kt:kt + 1], scalar2=b[:, kt:kt + 1],
                        op0=ALU.mult, op1=ALU.add)

        _mark(nc, "B:ln1")
        # ================= Phase B: LN1 ====================================
        with tc.tile_pool(name="ps_row1", bufs=4, space="PSUM") as ps_row, \
             tc.tile_pool(name="ps_bc1", bufs=4, space="PSUM") as ps_bc:
            layernorm_T(xT, NSLOT, 2, params["g1T"], params["b1T"], hT,
                        (ps_row, ps_bc))

        _mark(nc, "C:qkv")
        # ================= Phase C: QKV (+ q2047 column) ===================
        with tc.tile_pool(name="wstr", bufs=5) as wstr, \
             tc.tile_pool(name="ps_big", bufs=6, space="PSUM") as ps_big, \
             tc.tile_pool(name="ps_tiny", bufs=2, space="PSUM") as ps_tiny:
            for m in range(8):
                msl = slice(m * 128, (m + 1) * 128)
                wqm = wstr.tile([128, 8, 128], BF16, tag="wqm")
                nc.sync.dma_start(out=wqm, in_=inp["wq"][:, :, msl])
                wkm = wstr.tile([128, 8, 128], BF16, tag="wkm")
                nc.sync.dma_start(out=wkm, in_=inp["wk"][:, :, msl])
                q_ps = ps_big.tile([128, CHUNK], F32, tag="big")
                q47_ps = ps_tiny.tile([128, 1], F32, tag="tiny")
                for kt in range(8):
                    nc.tensor.matmul(q_ps, wqm[:, kt, :],
                                     hT[:, kt, HALO:HALO + CHUNK],
                                     start=kt == 0, stop=kt == 7)
                    nc.tensor.matmul(q47_ps, wqm[:, kt, :], hT[:, kt, 641:642],
                                     start=kt == 0, stop=kt == 7)
                nc.scalar.mul(out=QT[:, m, :], in_=q_ps, mul=1.0 / np.sqrt(HD))
                nc.scalar.mul(out=q47T[:, m:m + 1], in_=q47_ps,
                              mul=1.0 / np.sqrt(HD))
                k_ps = ps_big.tile([128, 512], F32, tag="big")
                k_ps2 = ps_big.tile([128, NKV - 512], F32, tag="big")
                for kt in range(8):
                    nc.tensor.matmul(k_ps, wkm[:, kt, :], hT[:, kt, 0:512],
                                     start=kt == 0, stop=kt == 7)
                    nc.tensor.matmul(k_ps2, wkm[:, kt, :], hT[:, kt, 512:NKV],
                                     start=kt == 0, stop=kt == 7)
                nc.scalar.copy(out=KT[:, m, 0:512], in_=k_ps)
                nc.scalar.copy(out=KT[:, m, 512:NKV], in_=k_ps2)
            for tt in range(6):
                for cch in range(2):
                    v_ps = ps_big.tile([128, 512], F32, tag="big")
                    for kt in range(8):
                        nc.tensor.matmul(
                            v_ps, hT[:, kt, tt * 128:(tt + 1) * 128],
                            wvsb[:, kt, cch * 512:(cch + 1) * 512],
                            start=kt == 0, stop=kt == 7)
                    nc.scalar.copy(out=V[:, tt, cch * 512:(cch + 1) * 512],
                                   in_=v_ps)
            _mark(nc, "D:partials")
            # ============= Phase D: global-row partials + AllReduce ========
            sT = small.tile([128, H * 4], F32, tag="p_sT")
            for h in range(H):
                p0 = 64 * (h % 2)
                s47_ps = ps_tiny.tile([128, 4], F32, tag="tiny")
                for i in range(4):
                    nc.tensor.matmul(
                        s47_ps[:, i:i + 1],
                        KT[p0:p0 + 64, h // 2,
                           HALO + 128 * i:HALO + 128 * (i + 1)],
                        q47T[p0:p0 + 64, h // 2:h // 2 + 1],
                        start=True, stop=True)
                nc.scalar.copy(out=sT[:, 4 * h:4 * h + 4], in_=s47_ps)
            p47 = small.tile([128, H * 4], BF16, tag="p_p47")
            nc.scalar.activation(out=p47, in_=sT, func=AF.Exp)
            ssum_ps = ps_tiny.tile([1, H * 4], F32, tag="tiny")
            nc.tensor.matmul(ssum_ps, onesb, p47, start=True, stop=True)
            s_c = small.tile([1, H], F32, tag="p_sc")
            nc.vector.reduce_sum(
                out=s_c, in_=ssum_ps.rearrange("p (h i) -> p h i", i=4),
                axis=AX.X)
            oall = small.tile([65, H], F32, tag="p_oall")
            for h in range(H):
                o47_ps = ps_tiny.tile([64, 1], F32, tag="tiny")
                for i in range(4):
                    nc.tensor.matmul(o47_ps, V[:, 1 + i, 64 * h:64 * h + 64],
                                     p47[:, 4 * h + i:4 * h + i + 1],
                                     start=i == 0, stop=i == 3)
                nc.scalar.copy(out=oall[0:64, h:h + 1], in_=o47_ps)
            nc.sync.dma_start(out=oall[64:65, :], in_=s_c)
            part_ps = ps_tiny.tile([H, 65], F32, tag="tiny")
            nc.tensor.transpose(part_ps, oall, idf[0:65, 0:65])
            part_sb = small.tile([H, 65], F32, tag="p_part")
            nc.scalar.copy(out=part_sb, in_=part_ps)
            pa = small.tile([H, 2, 65], F32, tag="p_pa")
            nc.vector.tensor_scalar_mul(out=pa[:, 0, :], in0=part_sb,
                                        scalar1=params["fA"])
            nc.vector.tensor_scalar_mul(out=pa[:, 1, :], in0=part_sb,
                                        scalar1=params["fB"])
            nc.sync.dma_start(out=pin[:], in_=pa)
            if not SKIP_CC[0]:
                nc.gpsimd.collective_compute(
                    "AllReduce", ALU.add,
                    replica_groups=[[0, 1, 2, 3, 4, 5, 6, 7]],
                    ins=[pin[:]], outs=[pout[:]])
            gath = small.tile([H, 2, 65], F32, tag="p_gath")
            nc.sync.dma_start(out=gath,
                              in_=(pin if SKIP_CC[0] else pout)[:])
            vA = small.tile([H, 65], F32, tag="p_vA")
            nc.vector.tensor_scalar_mul(out=vA, in0=gath[:, 0, :],
                                        scalar1=params["fA"])
            vB = small.tile([H, 65], F32, tag="p_vB")
            nc.vector.tensor_scalar_mul(out=vB, in0=gath[:, 1, :],
                                        scalar1=params["fB"])
            val = small.tile([H, 65], F32, tag="p_val")
            nc.vector.tensor_add(out=val, in0=vA, in1=vB)
            recS = small.tile([H, 1], F32, tag="p_recS")
            nc.vector.reciprocal(out=recS, in_=val[:, 64:65])
            a47 = small.tile([H, HD], F32, tag="p_a47")
            nc.vector.tensor_scalar_mul(out=a47, in0=val[:, 0:64],
                                        scalar1=recS)
            a47t_ps = ps_tiny.tile([HD, H], F32, tag="tiny")
            nc.tensor.transpose(a47t_ps, a47, idf[0:H, 0:H])
            a47T = small.tile([HD, H], BF16, tag="p_a47T")
            nc.scalar.copy(out=a47T, in_=a47t_ps)
            fix_sb = small.tile([128, 8], BF16, tag="p_fix")
            a47v = a47T.rearrange("p (t two) -> p t two", two=2)
            nc.sync.dma_start(out=fix_sb[0:64, :], in_=a47v[:, :, 0])
            nc.sync.dma_start(out=fix_sb[64:128, :], in_=a47v[:, :, 1])



        _mark(nc, "E:attn")
        # ================= Phase E: windowed attention =====================
        with tc.tile_pool(name="ps_s", bufs=2, space="PSUM") as ps_s, \
             tc.tile_pool(name="ps_pt", bufs=3, space="PSUM") as ps_pt, \
             tc.tile_pool(name="ps_p0", bufs=1, space="PSUM") as ps_p0, \
             tc.tile_pool(name="ps_o", bufs=2, space="PSUM") as ps_o:
            for pr in range(8):
                for qb in range(NQB):
                    o_ps = ps_o.tile([128, 128], F32, tag="o")
                    for sub in range(2):
                        h = 2 * pr + sub
                        p0 = 64 * sub
                        qs = QT[p0:p0 + 64, pr, qb * 128:(qb + 1) * 128]
                        s_ps = ps_s.tile([128, WIN + 1], F32, tag="s")
                        nc.tensor.matmul(
                            s_ps[:, 0:WIN], qs,
                            KT[p0:p0 + 64, pr, qb * 128:qb * 128 + WIN],
                            start=True, stop=False)
                        nc.tensor.matmul(s_ps[:, WIN:WIN + 1], qs,
                                         KT[p0:p0 + 64, pr, 640:641],
                                         start=False, stop=False)
                        nc.tensor.matmul(s_ps, idb, msk[:, qb, :],
                                         start=False, stop=True)
                        p = small.tile([128, WIN + 1], BF16, tag="a_p")
                        rsum = small.tile([128, 1], F32, tag="a_rsum")
                        nc.scalar.activation(out=p, in_=s_ps, func=AF.Exp,
                                             bias=neg3, scale=1.0,
                                             accum_out=rsum)
                        recip = small.tile([128, 1], F32, tag="a_recip")
                        nc.vector.reciprocal(out=recip, in_=rsum)
                        p2 = small.tile([128, WIN + 1], BF16, tag="a_p2")
                        nc.vector.tensor_scalar_mul(out=p2, in0=p,
                                                    scalar1=recip)
                        pt_ps = ps_pt.tile([128, WIN], BF16, tag="pt")
                        nc.tensor.transpose(pt_ps[:, 0:128], p2[:, 0:128], idb)
                        nc.tensor.transpose(pt_ps[:, 128:256], p2[:, 128:256],
                                            idb)
                        ptb = small.tile([128, WIN], BF16, tag="a_ptb")
                        if sub == 0:
                            nc.scalar.copy(out=ptb, in_=pt_ps)
                        else:
                            nc.vector.tensor_copy(out=ptb, in_=pt_ps)
                        pt0_ps = ps_p0.tile([1, 128], BF16, tag="pt0")
                        nc.tensor.transpose(pt0_ps, p2[:, WIN:WIN + 1], idb)
                        pt0b = small.tile([1, 128], BF16, tag="a_pt0b")
                        nc.vector.tensor_copy(out=pt0b, in_=pt0_ps)
                        dv = slice(64 * h, 64 * h + 64)
                        nc.tensor.matmul(o_ps[p0:p0 + 64, :], V[:, qb, dv],
                                         ptb[:, 0:128], start=True, stop=False)
                        nc.tensor.matmul(o_ps[p0:p0 + 64, :], V[:, qb + 1, dv],
                                         ptb[:, 128:256], start=False,
                                         stop=False)
                        nc.tensor.matmul(o_ps[p0:p0 + 64, :], V[0:1, 5, dv],
                                         pt0b, start=False, stop=True)
                    nc.vector.tensor_copy(
                        out=OT[:, pr, qb * 128:(qb + 1) * 128], in_=o_ps)

        _mark(nc, "F:patch")
        # ================= Phase F: patch global row =======================
        for t in range(8):
            nc.vector.copy_predicated(out=OT[:, t, CHUNK - 1:CHUNK],
                                      mask=params["fixsel"],
                                      data=fix_sb[:, t:t + 1])

        _mark(nc, "G:wo")
        # ================= Phase G: out-proj + residual ====================
        with tc.tile_pool(name="wostr", bufs=4) as wostr, \
             tc.tile_pool(name="ps_g", bufs=4, space="PSUM") as ps_g:
            for m in range(8):
                wom = wostr.tile([128, 8, 128], BF16, tag="wom")
                nc.sync.dma_start(out=wom,
                                  in_=inp["wo"][:, :, m * 128:(m + 1) * 128])
                pr_ps = ps_g.tile([128, CHUNK], F32, tag="g")
                for kt in range(8):
                    nc.tensor.matmul(pr_ps, wom[:, kt, :], OT[:, kt, :],
                                     start=kt == 0, stop=kt == 7)
                y1 = small.tile([128, CHUNK], F32, tag="evac512")
                nc.scalar.activation(out=y1, in_=pr_ps, func=AF.Identity,
                                     bias=params["boT"][:, m:m + 1], scale=1.0)
                nc.vector.tensor_add(out=yT[:, m, :], in0=y1,
                                     in1=xT[:, m, HALO:HALO + CHUNK])

        _mark(nc, "H:ln2")
        # ================= Phase H: LN2 ====================================
        with tc.tile_pool(name="ps_row2", bufs=2, space="PSUM") as ps_row2, \
             tc.tile_pool(name="ps_bc2", bufs=2, space="PSUM") as ps_bc2:
            layernorm_T(yT, CHUNK, 1, params["g2T"], params["b2T"], h2T,
                        (ps_row2, ps_bc2))

        _mark(nc, "I:ffn1")
        # ================= Phase I: FFN1 + gelu (spill HT to DRAM) =========
        with tc.tile_pool(name="w1p", bufs=6) as w1p, \
             tc.tile_pool(name="ps_f1", bufs=4, space="PSUM") as ps_f1:
            for m in range(32):
                w1t = w1p.tile([128, 8, 128], BF16, tag="w1t")
                nc.sync.dma_start(out=w1t, in_=inp["w1"][:, m, :, :])
                h_ps = ps_f1.tile([128, CHUNK], F32, tag="f1")
                for kt in range(8):
                    nc.tensor.matmul(h_ps, w1t[:, kt, :], h2T[:, kt, :],
                                     start=kt == 0, stop=kt == 7)
                htm = small.tile([128, CHUNK], BF16, tag="ht_m")
                nc.scalar.activation(out=htm, in_=h_ps, func=AF.Gelu,
                                     bias=params["b1h"][:, m:m + 1], scale=1.0)
                nc.sync.dma_start(out=ht_d[m], in_=htm)

        _mark(nc, "J:ffn2")
        # ================= Phase J: FFN2 + residual + out ==================
        with tc.tile_pool(name="ps_f2", bufs=1, space="PSUM") as ps_f2, \
             tc.tile_pool(name="w2p", bufs=8) as w2p, \
             tc.tile_pool(name="htp", bufs=6) as htp:
            f2_ps = ps_f2.tile([128, 8, CHUNK], F32, tag="f2")
            for kt in range(32):
                w2t = w2p.tile([128, D], BF16, tag="w2t")
                nc.sync.dma_start(out=w2t, in_=inp["w2"][kt])
                htk = htp.tile([128, CHUNK], BF16, tag="htk")
                nc.sync.dma_start(out=htk, in_=ht_d[kt])
                for m in range(8):
                    nc.tensor.matmul(f2_ps[:, m, :],
                                     w2t[:, m * 128:(m + 1) * 128],
                                     htk, start=kt == 0, stop=kt == 31)
            for m in range(8):
                f1 = small.tile([128, CHUNK], F32, tag="evac512")
                nc.scalar.activation(out=f1, in_=f2_ps[:, m, :],
                                     func=AF.Identity,
                                     bias=params["bo2T"][:, m:m + 1],
                                     scale=1.0)
                om = small.tile([128, CHUNK], F32, tag="out_m")
                nc.vector.tensor_add(out=om, in0=f1, in1=yT[:, m, :])
                nc.sync.dma_start(out=out_d[:, m, :], in_=om)

# ------------------------------------------------------------------ driver --

_CACHE = {}


def _prep_core_inputs(inputs, c, shared_cache={}):
    bf = ml_dtypes.bfloat16
    key = id(inputs.get("Wq"))
    shared = shared_cache.get(key)
    if shared is None:
        shared_cache.clear()
        shared = {
            "wq": _tileP(np.asarray(inputs["Wq"], np.float32).astype(bf)),
            "wk": _tileP(np.asarray(inputs["Wk"], np.float32).astype(bf)),
            "wv": _tileP(np.asarray(inputs["Wv"], np.float32).astype(bf)),
            "wo": _tileP(np.asarray(inputs["Wo"], np.float32).astype(bf)),
            "w1": np.ascontiguousarray(
                np.asarray(inputs["W1"], np.float32).astype(bf)
                .reshape(8, 128, 32, 128).transpose(1, 2, 0, 3)),
            "w2": np.ascontiguousarray(
                np.asarray(inputs["W2"], np.float32).astype(bf)
                .reshape(32, 128, D)),
            "g1T": _vec_t(inputs["ln1_g"]), "b1T": _vec_t(inputs["ln1_b"]),
            "g2T": _vec_t(inputs["ln2_g"]), "b2T": _vec_t(inputs["ln2_b"]),
            "boT": _vec_t(inputs["bo"]), "bo2T": _vec_t(inputs["b2"]),
            "b1h": np.ascontiguousarray(
                np.asarray(inputs["b1"], np.float32).reshape(32, 128).T),
        }
        shared_cache[key] = shared
    x = np.asarray(inputs["x"], np.float32)
    xT = np.ascontiguousarray(
        _make_x_ext(x, c).T.reshape(8, 128, NSLOT).transpose(1, 0, 2))
    msk = np.ascontiguousarray(
        _make_mask(c).transpose(1, 0, 2)).astype(ml_dtypes.bfloat16)
    fs = np.full((128, 1), 1 if c % 4 == 3 else 0, np.uint8)
    fA = np.full((16, 1), 1.0 if c < 4 else 0.0, np.float32)
    fB = np.full((16, 1), 0.0 if c < 4 else 1.0, np.float32)
    return {**shared, "xT": xT, "msk": msk, "fixsel": fs, "fA": fA, "fB": fB}


def get_nc():
    if "nc" not in _CACHE:
        _CACHE["nc"] = _build_nc()
    return _CACHE["nc"]


def kernel(**inputs):
    nc = get_nc()
    in_maps = [_prep_core_inputs(inputs, c) for c in range(N_CORES)]
    res = run_bass_kernel_spmd(nc, in_maps, core_ids=list(range(N_CORES)),
                               trace=False)
    out = np.zeros((B, T, D), np.float32)
    for c in range(N_CORES):
        b, j = divmod(c, 4)
        oT = res.results[c]["outT"]          # [128, 8, 512]
        out[b, j * CHUNK:(j + 1) * CHUNK] = \
            oT.transpose(1, 0, 2).reshape(D, CHUNK).T
    return out



# revision 81
# speedup vs baseline: 1.5448x; 1.5448x over previous
"""Longformer block on 8 TRN2 NeuronCores (Bass/Tile, SPMD), fp8 edition.

Sharding: data-parallel over (batch, sequence): core c -> batch c//4, token
chunk (c%4)*512..+512. Weights replicated (fp8, host-quantized). On-chip
layout is transposed [D, token] so LN/residual/matmuls need no device
transposes (LN stats via ones-vector matmuls on PE).

Quantization scheme (all scales powers of two; products land on one PSUM
scale per GEMM so a single accumulation suffices):
  h  = LN1 out, fp8 q(32 h);  W{q,k,v,o} fp8 q(2048 W)   -> psum 2^16
  V stored fp8 q(32 v); attention P stored fp8 q(8 exp(s-3)) (=exp(s+ln8-3))
  OT fp8 q(32 o) via reciprocal-fold;  Wo psum 2^16
  h2 split: hi=q(16 h2), lo=q(16 h2 - hi);  W1 split hi=q(2048 W1),
    lo=q(2048 (W1-hi/2048))  -> all FFN1 products at psum 2^15
    (3 groups: hi*W1hi, hi*W1lo, lo*W1hi; the dropped lo*W1lo term is
     O(2^-8) relative)
  g1 = gelu out, q(32 g1); W2 split likewise at 2048 -> psum 2^16; optional
    third group g1lo*W2hi (F2_3TERM) for extra margin.
All big GEMMs run MatmulPerfMode.DoubleRow (2 k-tiles/instr, 0.5 cyc/row).

Attention is computed in transposed (key-major) layout: scores S^T[k,q]
directly from stationary K-slices, exp+binary-mask produce P^T fp8 in a
[qb][2 blocks][128] layout, row-sums and P^T V via fp8 DoubleRow matmuls,
normalization folded into the PSUM->SBUF evacuation (no PE transposes).
Global token-0 column rides as a separate [1,512] score row; the global
row (q=T-1) uses per-core exp-sum partials + a tiny AllReduce, patched in
with copy_predicated (as before).
"""

import numpy as np
import ml_dtypes

import concourse.bass as bass
import concourse.mybir as mybir
import concourse.tile as tile
from concourse.bass_utils import run_bass_kernel_spmd

F32 = mybir.dt.float32
BF16 = mybir.dt.bfloat16
F8 = mybir.dt.float8e4
AF = mybir.ActivationFunctionType
ALU = mybir.AluOpType
AX = mybir.AxisListType
DR = mybir.MatmulPerfMode.DoubleRow
E4 = ml_dtypes.float8_e4m3

D = 1024
H = 16
HD = 64
T = 2048
B = 2
CHUNK = 512
HALO = 128
NSLOT = 768          # [halo 128 | own 512 | t2047 | t0 | pad]
NQ = 513             # Q cols: 512 own + q2047
NK = 642             # K cols: 0..640 slots + t0 at 641
NQB = 4
EPS = 1e-5
N_CORES = 8
SA = 32.0            # h, g1, V, OT activation scale
SA2 = 16.0           # h2 scale
SW = 2048.0          # weight scale
EV = 2.0 ** -16      # evac scale for 2^16 psums
EV1 = 2.0 ** -15     # evac scale for FFN1 (16*2048)
QSC = EV / np.sqrt(HD)
EXPB = float(np.log(8.0) - 3.0)   # P = exp(s + ln8 - 3) = 8 exp(s-3)
F2_3TERM = False     # third FFN2 group (g1lo * W2hi) for extra precision
SKIP_CC = [False]    # set True to build without the collective
DEBUG_DUMP = [False]  # dump intermediates to DRAM outputs
PHASE_MARKS = []


def _mark(nc, name):
    PHASE_MARKS.append((name, set(nc.inst_map.keys())))

# ---------------------------------------------------------------- bir fix ---

_waitfix_ctr = [0]


def _split_multiwaits(nc):
    """This container's walrus accepts ONE sync-wait per instruction; Tile
    attaches several. Hoist extras onto NoOps just before each instruction."""
    n = 0
    for func in nc.m.functions:
        for bb in func.blocks:
            out = []
            changed = False
            for inst in bb.instructions:
                si = inst.sync_info
                if si is not None and len(si.on_wait) > 1:
                    waits = list(si.on_wait)
                    keep = [w for w in waits
                            if getattr(w, "wait_mode", "") not in
                            ("sem-ge-imm", "sem-ge-reg")]
                    if keep:
                        hoist = [w for w in waits if w not in keep]
                        last = keep
                    else:
                        hoist, last = waits[:-1], [waits[-1]]
                    for w in hoist:
                        _waitfix_ctr[0] += 1
                        nop = mybir.InstNoOp(name=f"I-waitfix-{_waitfix_ctr[0]}")
                        nop.engine = inst.engine
                        nop.sync_info = mybir.SyncInfo(on_wait=[w], on_update=[])
                        out.append(nop)
                        n += 1
                    si.on_wait = last
                    changed = True
                out.append(inst)
            if changed:
                bb.instructions[:] = out
    return n

# ------------------------------------------------------------ host helpers --


def _q8(a, s):
    return (np.asarray(a, np.float32) * s).astype(E4)


def _make_x_ext(x, c):
    b, j = divmod(c, 4)
    start = j * CHUNK
    ext = np.zeros((NSLOT, D), np.float32)
    ext[0:HALO] = x[b, start - HALO:start] if j > 0 else x[b, 0:HALO]
    ext[HALO:HALO + CHUNK] = x[b, start:start + CHUNK]
    ext[640] = x[b, 0]
    ext[641] = x[b, T - 1]
    return ext


def _make_maskt(c):
    """[128, 4, 2, 128] additive bf16 mask (0 valid, -1e30 invalid): slot
    (i,0)=block i (jl>=il), slot (i,1)=block i+1 (jl<=il). Block 0 is the
    halo: invalid on j==0 cores."""
    b, j = divmod(c, 4)
    jl = np.arange(128)[:, None]
    il = np.arange(128)[None, :]
    m = np.full((128, NQB, 2, 128), -1e30, np.float32)
    for i in range(NQB):
        lo = (jl >= il)
        if i == 0 and j == 0:
            lo = np.zeros_like(lo)
        m[:, i, 0, :] = np.where(lo, 0.0, -1e30)
        m[:, i, 1, :] = np.where(jl <= il, 0.0, -1e30)
    return m.astype(ml_dtypes.bfloat16)


def _make_mask0(c):
    b, j = divmod(c, 4)
    row = np.zeros((1, CHUNK), np.float32)
    if j == 0:
        row[0, :HALO + 1] = -1e30
    return row.astype(ml_dtypes.bfloat16)


def _tileP(a, p=128):
    """[N*p, ...] -> [p, N, ...] partition-tiled layout."""
    n = a.shape[0] // p
    return np.ascontiguousarray(
        a.reshape(n, p, *a.shape[1:]).transpose(1, 0, *range(2, a.ndim + 1)))


def _vec_t(v):
    return np.ascontiguousarray(np.asarray(v, np.float32).reshape(-1, 128).T)

# ------------------------------------------------------------ bass program --


def _build_nc():
    nc = bass.Bass()

    inp = {}
    for name, shape, dt in [
        ("xT", [128, 8, NSLOT], F32),
        ("wq8", [128, 8, D], F8), ("wk8", [128, 8, D], F8),
        ("wv8", [128, 8, D], F8), ("wo8", [128, 8, D], F8),
        ("w1q", [128, 32, 8, 2, 128], F8),
        ("w2q", [32, 128, 2, D], F8),
        ("maskT", [128, NQB, 2, 128], BF16), ("mask0", [1, CHUNK], BF16),
        ("pk", [128, 96], F32),
        ("fixsel", [128, 1], mybir.dt.uint8),
        ("fA", [16, 1], F32), ("fB", [16, 1], F32),
    ]:
        inp[name] = nc.dram_tensor(name, shape, dt, kind="ExternalInput")
    out_d = nc.dram_tensor("outT", [128, 8, CHUNK], F32, kind="ExternalOutput")
    pin = nc.dram_tensor("pin", [H, 2, HD + 1], F32)
    pout = nc.dram_tensor("pout", [H, 2, HD + 1], F32, addr_space="Shared")
    dbg = {}
    if DEBUG_DUMP[0]:
        for nm, shape, dt in [
            ("d_hT8", [128, 8, NSLOT], F8), ("d_QT", [128, 8, NQ], BF16),
            ("d_KT", [128, 8, NK], BF16), ("d_V8", [128, 6, D], F8),
            ("d_OT8", [128, 8, CHUNK], F8), ("d_yT", [128, 8, CHUNK], F32),
            ("d_h2hi", [128, 8, CHUNK], F8), ("d_h2lo", [128, 8, CHUNK], F8),
            ("d_ht8", [128, 32, CHUNK], F8),
            ("d_P8s0", [128, 1024], F8), ("d_P8s1", [128, 1024], F8),
            ("d_P0s0", [1, CHUNK], F8),
            ("d_rcp0", [128, CHUNK], F32),
        ]:
            dbg[nm] = nc.dram_tensor(nm, shape, dt, kind="ExternalOutput")

    with tile.TileContext(nc) as tc:
        _emit(nc, tc, inp, out_d, pin, pout, dbg)
    _split_multiwaits(nc)
    return nc


def _emit(nc, tc, inp, out_d, pin, pout, dbg=None):
    from contextlib import ExitStack
    ctx = ExitStack()
    with ctx:
        pers = ctx.enter_context(tc.tile_pool(name="pers", bufs=1))
        small = ctx.enter_context(tc.tile_pool(name="small", bufs=3))
        lnb = ctx.enter_context(tc.tile_pool(name="lnb", bufs=2))
        big = ctx.enter_context(tc.tile_pool(name="big", bufs=1))

        # ---- input activations first: LN1 waits on these
        xT = big.tile([128, 8, NSLOT], F32, tag="xT")
        for kt in range(8):
            nc.sync.dma_start(out=xT[:, kt, :], in_=inp["xT"][:, kt, :])

        # ---- persistent constants / params
        onesD = pers.tile([128, 1], BF16, tag="onesD")   # 1/D for means
        nc.vector.memset(onesD, 1.0 / D)
        # Half-ones DR stationaries: sub 0 sums land on partitions 0..64,
        # sub 1 on 64..128, sharing one PSUM accumulation (zero columns
        # contribute zero to the other half).
        ones8 = [pers.tile([128, 2, 128], F8, tag=f"ones8{s}",
                           name=f"ones8{s}") for s in range(2)]
        for s in range(2):
            nc.vector.memset(ones8[s], 0.0)
            nc.vector.memset(ones8[s][:, :, 64 * s:64 * s + 64], 1.0)
        # [ones-half | zeros] tok0-row stationaries: the tok0 row rides a
        # DoubleRow matmul (mixing DR and regular matmuls in one PSUM
        # accumulation group drops the DR partials)
        onesz = [pers.tile([1, 2, 128], F8, tag=f"onesz{s}",
                           name=f"onesz{s}") for s in range(2)]
        for s in range(2):
            nc.vector.memset(onesz[s], 0.0)
            nc.vector.memset(onesz[s][:, 0, 64 * s:64 * s + 64], 1.0)
        vz = pers.tile([1, 2, D], F8, tag="vz")
        nc.vector.memset(vz, 0.0)
        one1b = pers.tile([1, 1], BF16, tag="one1b")
        nc.vector.memset(one1b, 1.0)
        onesb8 = pers.tile([128, 1], F8, tag="onesb8")
        nc.vector.memset(onesb8, 1.0)
        ones1f = pers.tile([1, 128], F32, tag="ones1f")
        nc.vector.memset(ones1f, 1.0)
        epst = pers.tile([1, 1], F32, tag="epst")
        nc.vector.memset(epst, EPS)
        expb = pers.tile([128, 1], F32, tag="expb")
        nc.vector.memset(expb, EXPB)
        idf = pers.tile([128, 128], F32, tag="idf")
        from concourse.masks import make_identity
        make_identity(nc, idf)
        idb = pers.tile([128, 128], BF16, tag="idb")
        make_identity(nc, idb)

        pk = pers.tile([128, 96], F32, tag="pk")
        nc.sync.dma_start(out=pk, in_=inp["pk"][:])
        params = {nm: pk[:, 8 * i:8 * (i + 1)] for i, nm in enumerate(
            ["g1T32", "b1T32", "g2T", "b2T", "g2T32", "b2T32", "boT",
             "bo2T"])}
        params["b1h"] = pk[:, 64:96]
        for nm in ["fixsel", "fA", "fB"]:
            dt = mybir.dt.uint8 if nm == "fixsel" else F32
            t = pers.tile(list(inp[nm].shape), dt, tag=nm, name=nm)
            nc.sync.dma_start(out=t, in_=inp[nm][:])
            params[nm] = t
        maskT = pers.tile([128, NQB, 2, 128], BF16, tag="maskT")
        nc.sync.dma_start(out=maskT, in_=inp["maskT"][:])
        maskTf = maskT.rearrange("p a b c -> p (a b c)")
        mask0 = pers.tile([1, CHUNK], BF16, tag="mask0")
        nc.sync.dma_start(out=mask0, in_=inp["mask0"][:])

        wv8 = big.tile([128, 8, D], F8, tag="wv8")
        nc.sync.dma_start(out=wv8, in_=inp["wv8"][:])
        hT8 = big.tile([128, 8, NSLOT], F8, tag="hT8")
        QT = big.tile([128, 8, NQ], BF16, tag="QT")
        KT = big.tile([128, 8, NK], BF16, tag="KT")
        V8 = big.tile([128, 6, D], F8, tag="V8")
        OT8 = big.tile([128, 8, CHUNK], F8, tag="OT8")
        yT = big.tile([128, 8, CHUNK], F32, tag="yT")
        h2f = big.tile([128, 8, CHUNK], BF16, tag="h2f")
        h2hi = big.tile([128, 8, CHUNK], F8, tag="h2hi")
        h2lo = big.tile([128, 8, CHUNK], F8, tag="h2lo")
        ht8 = big.tile([128, 32, CHUNK], F8, tag="ht8")
        htlo = None
        if F2_3TERM:
            htlo = big.tile([128, 32, CHUNK], F8, tag="htlo", name="htlo")
        xbt = big.tile([128, 8, NSLOT], BF16, tag="ln_xb")

        # ================= LN in transposed layout =========================
        def layernorm_T(src, width, nchunks, writer, pools):
            ps_row, ps_bc = pools
            cw = width // nchunks
            mus = []
            for cch in range(nchunks):
                mus.append((ps_row.tile([1, cw], F32, tag="row", name="mu"),
                            ps_row.tile([1, cw], F32, tag="row", name="msq")))
            for kt in range(8):
                xb = xbt[:, kt, 0:width]
                xsq = small.tile([128, width], BF16, tag="ln_xsq")
                if kt % 2 == 0:
                    nc.scalar.copy(out=xb, in_=src[:, kt, :])
                else:
                    nc.gpsimd.tensor_copy(out=xb, in_=src[:, kt, :])
                nc.vector.tensor_mul(out=xsq, in0=src[:, kt, :],
                                     in1=src[:, kt, :])
                for cch in range(nchunks):
                    sl = slice(cch * cw, (cch + 1) * cw)
                    nc.tensor.matmul(mus[cch][0], onesD, xb[:, sl],
                                     start=kt == 0, stop=kt == 7)
                    nc.tensor.matmul(mus[cch][1], onesD, xsq[:, sl],
                                     start=kt == 0, stop=kt == 7)
            bcs = []
            for cch in range(nchunks):
                mu_ps, msq_ps = mus[cch]
                musb = small.tile([1, cw], F32, tag="ln_mu")
                nc.scalar.copy(out=musb, in_=mu_ps)
                tmp = small.tile([1, cw], F32, tag="ln_tmp")
                nc.vector.tensor_mul(out=tmp, in0=musb, in1=musb)
                nc.vector.tensor_sub(out=tmp, in0=msq_ps, in1=tmp)
                nc.scalar.activation(out=tmp, in_=tmp, func=AF.Sqrt,
                                     bias=epst, scale=1.0)
                nc.vector.reciprocal(out=tmp, in_=tmp)       # rstd
                nc.vector.tensor_mul(out=musb, in0=musb, in1=tmp)
                nc.scalar.mul(out=musb, in_=musb, mul=-1.0)  # -mu*rstd
                rb_ps = ps_bc.tile([128, cw], F32, tag="bc", name="rb")
                nc.tensor.matmul(rb_ps, ones1f, tmp, start=True, stop=True)
                nb_ps = ps_bc.tile([128, cw], F32, tag="bc", name="nb")
                nc.tensor.matmul(nb_ps, ones1f, musb, start=True, stop=True)
                rb_sb = small.tile([128, cw], BF16, tag="ln_rb")
                nc.scalar.copy(out=rb_sb, in_=rb_ps)
                nb_sb = small.tile([128, cw], BF16, tag="ln_nb")
                nc.scalar.copy(out=nb_sb, in_=nb_ps)
                bcs.append((rb_sb, nb_sb))
            t1Bs = []
            for cch in range(nchunks):
                sl = slice(cch * cw, (cch + 1) * cw)
                rb_sb, nb_sb = bcs[cch]
                rbB = (rb_sb.rearrange("p (a c) -> p a c", a=1)
                       .broadcast_to([128, 8, cw]))
                nbB = (nb_sb.rearrange("p (a c) -> p a c", a=1)
                       .broadcast_to([128, 8, cw]))
                t1B = lnb.tile([128, 8, cw], BF16, tag="ln_t1B")
                nc.vector.tensor_mul(out=t1B, in0=xbt[:, :, sl], in1=rbB)
                nc.vector.tensor_add(out=t1B, in0=t1B, in1=nbB)
                t1Bs.append(t1B)
            for kt in range(8):      # kt-major so consumers start early
                for cch in range(nchunks):
                    sl = slice(cch * cw, (cch + 1) * cw)
                    writer(kt, sl, t1Bs[cch][:, kt, :])

        _mark(nc, "B:ln1")
        # ================= Phase B: LN1 -> hT8 (fp8, scale 32) =============
        def ln1_writer(kt, sl, t1):
            eng = nc.gpsimd if kt % 2 == 0 else nc.vector
            eng.tensor_scalar(
                out=hT8[:, kt, sl], in0=t1,
                scalar1=params["g1T32"][:, kt:kt + 1],
                scalar2=params["b1T32"][:, kt:kt + 1],
                op0=ALU.mult, op1=ALU.add)

        with tc.tile_pool(name="ps_row1", bufs=4, space="PSUM") as ps_row, \
             tc.tile_pool(name="ps_bc1", bufs=4, space="PSUM") as ps_bc:
            layernorm_T(xT, NSLOT, 2, ln1_writer, (ps_row, ps_bc))

        _mark(nc, "C:qkv")
        # ================= Phase C: QKV (fp8 DoubleRow) ====================
        with tc.tile_pool(name="wstr", bufs=5) as wstr, \
             tc.tile_pool(name="ps_big", bufs=5, space="PSUM") as ps_big, \
             tc.tile_pool(name="ps_tiny", bufs=2, space="PSUM") as ps_tiny:
            for m in range(8):
                msl = slice(m * 128, (m + 1) * 128)
                wqm = wstr.tile([128, 8, 128], F8, tag="wqm")
                nc.sync.dma_start(out=wqm, in_=inp["wq8"][:, :, msl])
                wkm = wstr.tile([128, 8, 128], F8, tag="wkm")
                nc.sync.dma_start(out=wkm, in_=inp["wk8"][:, :, msl])
                q_ps = ps_big.tile([128, CHUNK], F32, tag="big")
                q47_ps = ps_tiny.tile([128, 1], F32, tag="tiny")
                for j in range(4):
                    kp = slice(2 * j, 2 * j + 2)
                    nc.tensor.matmul(q_ps, wqm[:, kp, :],
                                     hT8[:, kp, HALO:HALO + CHUNK],
                                     start=j == 0, stop=j == 3, perf_mode=DR)
                    nc.tensor.matmul(q47_ps, wqm[:, kp, :],
                                     hT8[:, kp, 641:642],
                                     start=j == 0, stop=j == 3, perf_mode=DR)
                nc.scalar.mul(out=QT[:, m, 0:CHUNK], in_=q_ps, mul=QSC)
                nc.scalar.mul(out=QT[:, m, CHUNK:NQ], in_=q47_ps, mul=QSC)
                k_ps = ps_big.tile([128, 512], F32, tag="big")
                k_ps2 = ps_big.tile([128, NK - 512], F32, tag="big")
                for j in range(4):
                    kp = slice(2 * j, 2 * j + 2)
                    nc.tensor.matmul(k_ps, wkm[:, kp, :], hT8[:, kp, 0:512],
                                     start=j == 0, stop=j == 3, perf_mode=DR)
                    nc.tensor.matmul(k_ps2, wkm[:, kp, :], hT8[:, kp, 512:NK],
                                     start=j == 0, stop=j == 3, perf_mode=DR)
                nc.vector.tensor_scalar(out=KT[:, m, 0:512], in0=k_ps,
                                        scalar1=EV, scalar2=None,
                                        op0=ALU.mult)
                nc.vector.tensor_scalar(out=KT[:, m, 512:NK], in0=k_ps2,
                                        scalar1=EV, scalar2=None,
                                        op0=ALU.mult)
            # cch-major, tile 5 first: head pairs 0..3 only need the cch=0
            # half (+ the tok0 vz row), so they unblock after 6 tiles
            for cch in range(2):
                for tt in (5, 0, 1, 2, 3, 4):
                    v_ps = ps_big.tile([128, 512], F32, tag="big")
                    for j in range(4):
                        kp = slice(2 * j, 2 * j + 2)
                        nc.tensor.matmul(
                            v_ps, hT8[:, kp, tt * 128:(tt + 1) * 128],
                            wv8[:, kp, cch * 512:(cch + 1) * 512],
                            start=j == 0, stop=j == 3, perf_mode=DR)
                    nc.vector.tensor_scalar(
                        out=V8[:, tt, cch * 512:(cch + 1) * 512], in0=v_ps,
                        scalar1=EV * SA, scalar2=None, op0=ALU.mult)
                    if tt == 5:
                        nc.scalar.copy(
                            out=vz[:, 0, cch * 512:(cch + 1) * 512],
                            in_=V8[0:1, 5, cch * 512:(cch + 1) * 512])
            _mark(nc, "D:partials")
            # ============= Phase D: global-row partials + AllReduce ========
            sT = small.tile([128, H * 4], F32, tag="p_sT")
            for h in range(H):
                p0 = 64 * (h % 2)
                s47_ps = ps_tiny.tile([128, 4], F32, tag="tiny")
                for i in range(4):
                    nc.tensor.matmul(
                        s47_ps[:, i:i + 1],
                        KT[p0:p0 + 64, h // 2,
                           HALO + 128 * i:HALO + 128 * (i + 1)],
                        QT[p0:p0 + 64, h // 2, CHUNK:NQ],
                        start=True, stop=True)
                nc.vector.tensor_copy(out=sT[:, 4 * h:4 * h + 4], in_=s47_ps)
            p47 = small.tile([128, H * 4], F8, tag="p_p47")
            nc.scalar.activation(out=p47, in_=sT, func=AF.Exp)
            ssum_ps = ps_tiny.tile([1, H * 4], F32, tag="tiny")
            nc.tensor.matmul(ssum_ps, onesb8, p47, start=True, stop=True)
            s_c = small.tile([1, H], F32, tag="p_sc")
            nc.vector.reduce_sum(
                out=s_c, in_=ssum_ps.rearrange("p (h i) -> p h i", i=4),
                axis=AX.X)
            oall = small.tile([65, H], F32, tag="p_oall")
            for h in range(H):
                o47_ps = ps_tiny.tile([64, 1], F32, tag="tiny")
                for i in range(4):
                    nc.tensor.matmul(o47_ps, V8[:, 1 + i, 64 * h:64 * h + 64],
                                     p47[:, 4 * h + i:4 * h + i + 1],
                                     start=i == 0, stop=i == 3)
                nc.vector.tensor_copy(out=oall[0:64, h:h + 1], in_=o47_ps)
            nc.sync.dma_start(out=oall[64:65, :], in_=s_c)
            part_ps = ps_tiny.tile([H, 65], F32, tag="tiny")
            nc.tensor.transpose(part_ps, oall, idf[0:65, 0:65])
            part_sb = small.tile([H, 65], F32, tag="p_part")
            nc.scalar.copy(out=part_sb, in_=part_ps)
            pa = small.tile([H, 2, 65], F32, tag="p_pa")
            nc.vector.tensor_scalar_mul(out=pa[:, 0, :], in0=part_sb,
                                        scalar1=params["fA"])
            nc.vector.tensor_scalar_mul(out=pa[:, 1, :], in0=part_sb,
                                        scalar1=params["fB"])
            nc.sync.dma_start(out=pin[:], in_=pa)
            if not SKIP_CC[0]:
                nc.gpsimd.collective_compute(
                    "AllReduce", ALU.add,
                    replica_groups=[[0, 1, 2, 3, 4, 5, 6, 7]],
                    ins=[pin[:]], outs=[pout[:]])
            gath = small.tile([H, 2, 65], F32, tag="p_gath")
            nc.sync.dma_start(out=gath,
                              in_=(pin if SKIP_CC[0] else pout)[:])
            vA = small.tile([H, 65], F32, tag="p_vA")
            nc.vector.tensor_scalar_mul(out=vA, in0=gath[:, 0, :],
                                        scalar1=params["fA"])
            vB = small.tile([H, 65], F32, tag="p_vB")
            nc.vector.tensor_scalar_mul(out=vB, in0=gath[:, 1, :],
                                        scalar1=params["fB"])
            val = small.tile([H, 65], F32, tag="p_val")
            nc.vector.tensor_add(out=val, in0=vA, in1=vB)
            recS = small.tile([H, 1], F32, tag="p_recS")
            nc.vector.reciprocal(out=recS, in_=val[:, 64:65])
            a47 = small.tile([H, HD], F32, tag="p_a47")
            nc.vector.tensor_scalar_mul(out=a47, in0=val[:, 0:64],
                                        scalar1=recS)
            a47t_ps = ps_tiny.tile([HD, H], F32, tag="tiny")
            nc.tensor.transpose(a47t_ps, a47, idf[0:H, 0:H])
            a47T = small.tile([HD, H], F8, tag="p_a47T")
            nc.scalar.copy(out=a47T, in_=a47t_ps)
            fix_sb = small.tile([128, 8], F8, tag="p_fix")
            a47v = a47T.rearrange("p (t two) -> p t two", two=2)
            nc.sync.dma_start(out=fix_sb[0:64, :], in_=a47v[:, :, 0])
            nc.sync.dma_start(out=fix_sb[64:128, :], in_=a47v[:, :, 1])

        _mark(nc, "E:attn")
        # ========= Phase E: windowed attention, transposed layout ==========
        with tc.tile_pool(name="ps_st", bufs=4, space="PSUM") as ps_st, \
             tc.tile_pool(name="ps_rs", bufs=1, space="PSUM") as ps_rs, \
             tc.tile_pool(name="ps_o", bufs=2, space="PSUM") as ps_o, \
             tc.tile_pool(name="ps_t0", bufs=1, space="PSUM") as ps_t0, \
             tc.tile_pool(name="p8p", bufs=4) as p8p, \
             tc.tile_pool(name="rcpp", bufs=2) as rcpp:
            for pr in range(8):
                # V8 column window for this head pair; head1's V sits at
                # cols 64..128 so its DR results land on partitions 64..127
                # (DR matmuls require dst base partition 0).
                dvw = slice(128 * pr, 128 * pr + 128)
                recipT = rcpp.tile([128, CHUNK], F32, tag="rcp")
                rs_ps = ps_rs.tile([128, CHUNK], F32, tag="rs")
                o_list = []
                for sub in range(2):
                    p0 = 64 * sub
                    P8 = p8p.tile([128, NQB, 2, 128], F8, tag="p8")
                    P8f = P8.rearrange("p a b c -> p (a b c)")
                    # scores S^T per k-block into paired psums; the additive
                    # band mask rides the same PSUM group via an identity
                    # matmul; exp covers two blocks at once
                    for g, kbs in enumerate(((0, 1), (2, 3), (4,))):
                        st = ps_st.tile([128, 512], F32, tag="st")
                        off = max(0, 128 * (2 * kbs[0] - 1))
                        tot = 0
                        for kb in kbs:
                            qlo = max(0, 128 * (kb - 1))
                            qhi = min(CHUNK, 128 * (kb + 1))
                            w = qhi - qlo
                            nc.tensor.matmul(
                                st[:, tot:tot + w],
                                KT[p0:p0 + 64, pr, 128 * kb:128 * (kb + 1)],
                                QT[p0:p0 + 64, pr, qlo:qhi],
                                start=kb == kbs[0], stop=False)
                            tot += w
                        nc.tensor.matmul(st[:, 0:tot], idb,
                                         maskTf[:, off:off + tot],
                                         start=False, stop=True)
                        nc.scalar.activation(out=P8f[:, off:off + tot],
                                             in_=st[:, 0:tot], func=AF.Exp,
                                             bias=expb, scale=1.0)
                    # token-0 global column as a [1, 512] score row; the
                    # additive row mask rides the PSUM group via 1x1 matmul
                    t0_ps = ps_t0.tile([1, CHUNK], F32, tag="t0")
                    nc.tensor.matmul(t0_ps, KT[p0:p0 + 64, pr, 640:641],
                                     QT[p0:p0 + 64, pr, 0:CHUNK],
                                     start=True, stop=False)
                    nc.tensor.matmul(t0_ps, one1b, mask0,
                                     start=False, stop=True)
                    P0 = p8p.tile([1, CHUNK], F8, tag="p0")
                    nc.scalar.activation(out=P0, in_=t0_ps, func=AF.Exp,
                                         bias=expb[0:1, :], scale=1.0)
                    P02 = (P0.rearrange("p (a c) -> p a c", a=1)
                           .broadcast_to([1, 2, CHUNK]))
                    # row sums (DR) + token-0 row; sub s lands on its own
                    # partition half of the shared accumulator
                    for i in range(NQB):
                        nc.tensor.matmul(
                            rs_ps[:, 128 * i:128 * (i + 1)],
                            ones8[sub], P8[:, i, :, :],
                            start=(sub == 0 and i == 0), stop=False,
                            perf_mode=DR)
                    nc.tensor.matmul(rs_ps, onesz[sub], P02,
                                     start=False, stop=sub == 1, perf_mode=DR)
                    # P^T V (DR) + token-0 row; rows p0..p0+64 are this head
                    o_ps = ps_o.tile([128, CHUNK], F32, tag="o")
                    o_list.append(o_ps)
                    for i in range(NQB):
                        nc.tensor.matmul(
                            o_ps[:, 128 * i:128 * (i + 1)],
                            V8[:, i:i + 2, dvw], P8[:, i, :, :],
                            start=i == 0, stop=False, perf_mode=DR)
                    nc.tensor.matmul(o_ps, vz[:, :, dvw], P02,
                                     start=False, stop=True, perf_mode=DR)
                    if dbg and pr == 0:
                        nc.sync.dma_start(out=dbg[f"d_P8s{sub}"][:], in_=P8f)
                        if sub == 0:
                            nc.sync.dma_start(out=dbg["d_P0s0"][:], in_=P0)
                nc.vector.reciprocal(out=recipT, in_=rs_ps)
                for sub in range(2):
                    p0 = 64 * sub
                    nc.vector.tensor_mul(out=OT8[p0:p0 + 64, pr, :],
                                         in0=o_list[sub][p0:p0 + 64, :],
                                         in1=recipT[p0:p0 + 64, :])
                if dbg and pr == 0:
                    nc.sync.dma_start(out=dbg["d_rcp0"][:], in_=recipT)

        _mark(nc, "F:patch")
        # ================= Phase F: patch global row =======================
        for t in range(8):
            nc.vector.copy_predicated(out=OT8[:, t, CHUNK - 1:CHUNK],
                                      mask=params["fixsel"],
                                      data=fix_sb[:, t:t + 1])

        _mark(nc, "G:wo")
        # ================= Phase G: out-proj + residual ====================
        with tc.tile_pool(name="wostr", bufs=4) as wostr, \
             tc.tile_pool(name="ps_g", bufs=4, space="PSUM") as ps_g:
            for m in range(8):
                wom = wostr.tile([128, 8, 128], F8, tag="wom")
                nc.sync.dma_start(out=wom,
                                  in_=inp["wo8"][:, :, m * 128:(m + 1) * 128])
                pr_ps = ps_g.tile([128, CHUNK], F32, tag="g")
                for j in range(4):
                    kp = slice(2 * j, 2 * j + 2)
                    nc.tensor.matmul(pr_ps, wom[:, kp, :], OT8[:, kp, :],
                                     start=j == 0, stop=j == 3, perf_mode=DR)
                y1 = small.tile([128, CHUNK], F32, tag="evac512")
                nc.scalar.activation(out=y1, in_=pr_ps, func=AF.Identity,
                                     bias=params["boT"][:, m:m + 1], scale=EV)
                nc.vector.tensor_add(out=yT[:, m, :], in0=y1,
                                     in1=xT[:, m, HALO:HALO + CHUNK])

        _mark(nc, "H:ln2")
        # ====== Phase H: LN2 -> h2f (bf16), h2hi=q(16 h2), h2lo ============
        def ln2_writer(kt, sl, t1):
            e1, e2 = ((nc.gpsimd, nc.vector) if kt % 2 == 0
                      else (nc.vector, nc.gpsimd))
            e1.tensor_scalar(
                out=h2f[:, kt, sl], in0=t1,
                scalar1=params["g2T"][:, kt:kt + 1],
                scalar2=params["b2T"][:, kt:kt + 1],
                op0=ALU.mult, op1=ALU.add)
            e2.tensor_scalar(
                out=h2hi[:, kt, sl], in0=t1,
                scalar1=params["g2T32"][:, kt:kt + 1],
                scalar2=params["b2T32"][:, kt:kt + 1],
                op0=ALU.mult, op1=ALU.add)
            nc.vector.scalar_tensor_tensor(
                out=h2lo[:, kt, sl], in0=h2f[:, kt, sl], scalar=SA2,
                in1=h2hi[:, kt, sl], op0=ALU.mult, op1=ALU.subtract)

        with tc.tile_pool(name="ps_row2", bufs=2, space="PSUM") as ps_row2, \
             tc.tile_pool(name="ps_bc2", bufs=2, space="PSUM") as ps_bc2:
            layernorm_T(yT, CHUNK, 1, ln2_writer, (ps_row2, ps_bc2))

        _mark(nc, "I:ffn1")
        # == Phase I: FFN1 3-term fp8 DR + gelu -> ht8, all in SBUF ========
        with tc.tile_pool(name="w1p", bufs=4) as w1p, \
             tc.tile_pool(name="ps_f1", bufs=4, space="PSUM") as ps_f1:
            for m in range(32):
                w1t = w1p.tile([128, 8, 2, 128], F8, tag="w1t")
                nc.sync.dma_start(out=w1t, in_=inp["w1q"][:, m])
                h_ps = ps_f1.tile([128, CHUNK], F32, tag="f1")
                for kt in range(8):
                    nc.tensor.matmul(
                        h_ps, w1t[:, kt, :, :],
                        h2hi[:, kt:kt + 1, :].broadcast_to([128, 2, CHUNK]),
                        start=kt == 0, stop=False, perf_mode=DR)
                for j in range(4):
                    kp = slice(2 * j, 2 * j + 2)
                    nc.tensor.matmul(h_ps, w1t[:, kp, 0, :], h2lo[:, kp, :],
                                     start=False, stop=j == 3, perf_mode=DR)
                g1m = small.tile([128, CHUNK], BF16, tag="g1m")
                nc.scalar.activation(out=g1m, in_=h_ps, func=AF.Gelu,
                                     bias=params["b1h"][:, m:m + 1], scale=EV1)
                nc.gpsimd.tensor_scalar(out=ht8[:, m, :], in0=g1m,
                                        scalar1=SA, scalar2=None, op0=ALU.mult)

        _mark(nc, "J:ffn2")
        # ================= Phase J: FFN2 fp8 DR + residual + out ===========
        with tc.tile_pool(name="ps_f2", bufs=1, space="PSUM") as ps_f2, \
             tc.tile_pool(name="w2p", bufs=5) as w2p:
            f2_ps = ps_f2.tile([128, 8, CHUNK], F32, tag="f2")
            for kt in range(32):
                w2t = w2p.tile([128, 2, D], F8, tag="w2t")
                eng = nc.sync if kt % 2 == 0 else nc.scalar
                eng.dma_start(out=w2t, in_=inp["w2q"][kt])
                rhs = ht8[:, kt:kt + 1, :].broadcast_to([128, 2, CHUNK])
                for m in range(8):
                    nc.tensor.matmul(f2_ps[:, m, :],
                                     w2t[:, :, m * 128:(m + 1) * 128],
                                     rhs, start=kt == 0, stop=kt == 31,
                                     perf_mode=DR)
            for m in range(8):
                f1 = small.tile([128, CHUNK], F32, tag="evac512")
                nc.scalar.activation(out=f1, in_=f2_ps[:, m, :],
                                     func=AF.Identity,
                                     bias=params["bo2T"][:, m:m + 1],
                                     scale=EV)
                om = small.tile([128, CHUNK], F32, tag="out_m")
                nc.vector.tensor_add(out=om, in0=f1, in1=yT[:, m, :])
                nc.sync.dma_start(out=out_d[:, m, :], in_=om)

        if dbg:
            for nm, t in [("d_hT8", hT8), ("d_QT", QT), ("d_KT", KT),
                          ("d_V8", V8), ("d_OT8", OT8), ("d_yT", yT),
                          ("d_h2hi", h2hi), ("d_h2lo", h2lo),
                          ("d_ht8", ht8)]:
                nc.sync.dma_start(out=dbg[nm][:], in_=t)

# ------------------------------------------------------------------ driver --

_CACHE = {}


def _prep_core_inputs(inputs, c, shared_cache={}):
    key = id(inputs.get("Wq"))
    shared = shared_cache.get(key)
    if shared is None:
        shared_cache.clear()
        W1 = np.asarray(inputs["W1"], np.float32)
        W1hi = _q8(W1, SW)
        W1lo = _q8(W1 - W1hi.astype(np.float32) / SW, SW)
        # [2, 8kt, 128k, 32m, 128mc] -> [128k, 32m, 8kt, 2, 128mc]
        w1q = np.ascontiguousarray(
            np.stack([W1hi, W1lo]).reshape(2, 8, 128, 32, 128)
            .transpose(2, 3, 1, 0, 4))
        W2 = np.asarray(inputs["W2"], np.float32)
        W2hi = _q8(W2, SW)
        W2lo = _q8(W2 - W2hi.astype(np.float32) / SW, SW)
        w2q = np.ascontiguousarray(
            np.stack([W2hi, W2lo]).reshape(2, 32, 128, D)
            .transpose(1, 2, 0, 3))
        pk = np.concatenate([
            _vec_t(np.asarray(inputs["ln1_g"], np.float32) * SA),
            _vec_t(np.asarray(inputs["ln1_b"], np.float32) * SA),
            _vec_t(inputs["ln2_g"]), _vec_t(inputs["ln2_b"]),
            _vec_t(np.asarray(inputs["ln2_g"], np.float32) * SA2),
            _vec_t(np.asarray(inputs["ln2_b"], np.float32) * SA2),
            _vec_t(inputs["bo"]), _vec_t(inputs["b2"]),
            np.asarray(inputs["b1"], np.float32).reshape(32, 128).T,
        ], axis=1).astype(np.float32)
        shared = {
            "wq8": _tileP(_q8(inputs["Wq"], SW)),
            "wk8": _tileP(_q8(inputs["Wk"], SW)),
            "wv8": _tileP(_q8(inputs["Wv"], SW)),
            "wo8": _tileP(_q8(inputs["Wo"], SW)),
            "w1q": w1q, "w2q": w2q,
            "pk": np.ascontiguousarray(pk),
        }
        shared_cache[key] = shared
    x = np.asarray(inputs["x"], np.float32)
    xT = np.ascontiguousarray(
        _make_x_ext(x, c).T.reshape(8, 128, NSLOT).transpose(1, 0, 2))
    fs = np.full((128, 1), 1 if c % 4 == 3 else 0, np.uint8)
    fA = np.full((16, 1), 1.0 if c < 4 else 0.0, np.float32)
    fB = np.full((16, 1), 0.0 if c < 4 else 1.0, np.float32)
    return {**shared, "xT": xT, "maskT": _make_maskt(c),
            "mask0": _make_mask0(c), "fixsel": fs, "fA": fA, "fB": fB}


def get_nc():
    if "nc" not in _CACHE:
        _CACHE["nc"] = _build_nc()
    return _CACHE["nc"]


def kernel(**inputs):
    nc = get_nc()
    in_maps = [_prep_core_inputs(inputs, c) for c in range(N_CORES)]
    res = run_bass_kernel_spmd(nc, in_maps, core_ids=list(range(N_CORES)),
                               trace=False)
    out = np.zeros((B, T, D), np.float32)
    for c in range(N_CORES):
        b, j = divmod(c, 4)
        oT = res.results[c]["outT"]          # [128, 8, 512]
        out[b, j * CHUNK:(j + 1) * CHUNK] = \
            oT.transpose(1, 0, 2).reshape(D, CHUNK).T
    return out
